# revision 1
# baseline (speedup 1.0000x reference)
"""Trainium2 Bass kernel for masked multi-modal causal dot-product attention.

Computation (reference):
  Q = mlp(x1, Wq)               # (4096, 64), 3 linear layers, relu between
  for m in 0..3:
    K_m = mlp(x_m, Wk[m])       # (4096, 64)
    mask_m[i,j] = t2_m[j] <= t1[i]   (timestamps sorted -> staircase mask)
    acc += ((Q @ K_m.T) * mask_m) @ x_m[:, :2]
  out = acc  # (1, 4096, 2)

Sharding: 8 cores = 4 modalities x 2 query-parity halves (queries interleaved
by 128-chunks for load balance). One SPMD program; per-core variation lives in
the input tensors. Host classifies key tiles (full/boundary/invisible) exactly
from the actual timestamps, quantified over all cores.

Perf: TRN2 PE streams 1 moving column/cycle only when the contraction dim is
128; K=64 matmuls run at half rate. The feature dim here is 64, so everything
is packed to K=128:
  - MLPs run on [top|bottom]-stacked halves with block-diagonal weights.
  - K^T is written by the final MLP layer into block-diagonal pair tiles
    (kTblk[:, pair, :]: even 64-chunk on partitions 0:64 / cols 0:64, odd
    chunk on 64:128 / 64:128, zeros elsewhere) via 3D strided APs.
  - Q^T is replicated onto both partition halves by a special final layer
    ([[W,W],[0,0]] / [[0,0],[W,W]] stationaries), so S^T pair tiles
    [128 keys, 512 queries] come from one K=128 matmul each (228ns).
  - AV contracts the 128 pair rows directly: out^T += V2blk^T @ S^T.
All matmuls f32r (fp32-class numerics, ~5e-4 rel err end to end).
"""

import os
import sys

import numpy as np

sys.path.insert(0, "/opt/trn_rl_repo")

T = 4096
D = 64
M = 4
NLIN = 3
NQ = 2048          # packed queries per core
CHUNK = 128        # keys per pair tile (64 even + 64 odd)
NPAIR = T // CHUNK  # 32 pair tiles
IBLK = 512         # query block (moving dim)
NBLK = NQ // IBLK  # 4 query blocks per core

LAST_RESULTS = None


def _build_program(J, F):
    """J[b]: pair tiles to process for query block b. F[b]: tiles < F[b] are
    fully visible (plain copy); F[b] <= jt < J[b] get the on-device mask."""
    import concourse.bacc as bacc
    import concourse.mybir as mybir
    import concourse.tile as tile

    f32 = mybir.dt.float32
    f32r = mybir.dt.float32r
    Relu = mybir.ActivationFunctionType.Relu
    Identity = mybir.ActivationFunctionType.Identity
    is_ge = mybir.AluOpType.is_ge
    add = mybir.AluOpType.add
    amax = mybir.AluOpType.max

    nc = bacc.Bacc("TRN2", target_bir_lowering=False, debug=False, num_devices=8)

    xqT = nc.dram_tensor("xqT", [128, NQ // 2], f32, kind="ExternalInput")
    xkT = nc.dram_tensor("xkT", [128, T // 2], f32, kind="ExternalInput")
    xkv = nc.dram_tensor("xkv", [128, NPAIR * 2], f32, kind="ExternalInput")
    xt2 = nc.dram_tensor("xt2", [128, NPAIR], f32, kind="ExternalInput")
    t1p = nc.dram_tensor("t1p", [1, NQ], f32, kind="ExternalInput")
    wq = nc.dram_tensor("wq", [128, 4 * 128], f32, kind="ExternalInput")
    bq = nc.dram_tensor("bq", [128, 4], f32, kind="ExternalInput")
    wk = nc.dram_tensor("wk", [128, NLIN * 128], f32, kind="ExternalInput")
    bk = nc.dram_tensor("bk", [128, NLIN], f32, kind="ExternalInput")
    out = nc.dram_tensor("out", [2, NQ], f32, kind="ExternalOutput")

    def rr(ap):
        return ap.bitcast(f32r)

    with tile.TileContext(nc) as tc:
        with (
            tc.tile_pool(name="const", bufs=1) as const,
            tc.tile_pool(name="hq", bufs=2) as hqp,
            tc.tile_pool(name="hk", bufs=2) as hkp,
            tc.tile_pool(name="spool", bufs=4) as spool,
            tc.tile_pool(name="mpool", bufs=3) as mpool,
            tc.tile_pool(name="ps_mlp", bufs=3, space="PSUM") as ps_mlp,
            tc.tile_pool(name="ps_s", bufs=3, space="PSUM") as ps_s,
            tc.tile_pool(name="ps_o", bufs=2, space="PSUM") as ps_o,
        ):
            # ---- inputs -> SBUF (weights first, x chunked for overlap)
            wq_sb = const.tile([128, 4, 128], f32r)
            nc.sync.dma_start(wq_sb[:], rr(wq[:]).rearrange("p (l e) -> p l e", l=4))
            bq_sb = const.tile([128, 4], f32)
            nc.sync.dma_start(bq_sb[:], bq[:])
            wk_sb = const.tile([128, NLIN, 128], f32r)
            nc.sync.dma_start(wk_sb[:], rr(wk[:]).rearrange("p (l e) -> p l e", l=NLIN))
            bk_sb = const.tile([128, NLIN], f32)
            nc.sync.dma_start(bk_sb[:], bk[:])
            xkv_sb = const.tile([128, NPAIR, 2], f32r)
            nc.sync.dma_start(xkv_sb[:], rr(xkv[:]).rearrange("p (c f) -> p c f", f=2))
            xt2_sb = const.tile([128, NPAIR], f32)
            nc.sync.dma_start(xt2_sb[:], xt2[:])
            t1b_sb = const.tile([CHUNK, NQ], f32)
            nc.sync.dma_start(t1b_sb[:], t1p[:].partition_broadcast(CHUNK))

            xqT_sb = const.tile([128, NQ // 2], f32r)
            for nb in range(NQ // 2 // IBLK):
                sl = slice(nb * IBLK, (nb + 1) * IBLK)
                nc.sync.dma_start(xqT_sb[:, sl], rr(xqT[:, sl]))
            xkT_sb = const.tile([128, T // 2], f32r)
            for nb in range(T // 2 // IBLK):
                sl = slice(nb * IBLK, (nb + 1) * IBLK)
                nc.sync.dma_start(xkT_sb[:, sl], rr(xkT[:, sl]))

            out_sb = const.tile([2, NQ], f32)

            # ---- blocked K^T target: pair tiles with block-diagonal layout
            kTblk = const.tile([128, NPAIR, CHUNK], f32r)
            zeros_sb = const.tile([128, NPAIR, 64], f32)
            nc.vector.memset(zeros_sb[:], 0.0)
            nc.vector.tensor_copy(kTblk[0:64, :, 64:128], zeros_sb[0:64])
            nc.scalar.copy(kTblk[64:128, :, 0:64], zeros_sb[64:128])
            qT2 = const.tile([128, NQ], f32r)

            # ---- stacked MLPs (block-diagonal weights, both halves at once)
            def epilogue(dst, ps, bias, layer, eng):
                if eng == "act":
                    func = Relu if layer < NLIN - 1 else Identity
                    nc.scalar.activation(dst, ps, func, bias=bias)
                elif layer < NLIN - 1:
                    nc.vector.tensor_scalar(dst, ps, bias, 0.0, op0=add, op1=amax)
                else:
                    nc.vector.tensor_scalar(dst, ps, bias, None, op0=add)

            def mlp_hidden(cur, w_sb, b_sb, pool, nt, layer, eng):
                nxt = pool.tile([128, nt], f32r, tag="h")
                for nb in range(nt // IBLK):
                    sl = slice(nb * IBLK, (nb + 1) * IBLK)
                    ps = ps_mlp.tile([128, IBLK], f32)
                    nc.tensor.matmul(
                        ps[:], w_sb[:, layer, :], cur[:, sl], start=True, stop=True
                    )
                    epilogue(nxt[:, sl], ps[:], b_sb[:, layer : layer + 1], layer, eng)
                return nxt

            hk, hq = xkT_sb, xqT_sb
            for layer in range(NLIN - 1):
                hk = mlp_hidden(hk, wk_sb, bk_sb, hkp, T // 2, layer, "act")
                hq = mlp_hidden(hq, wq_sb, bq_sb, hqp, NQ // 2, layer, "dve")

            # final K layer: write straight into block-diagonal pair tiles
            eng_flip = 0
            for nb in range(T // 2 // IBLK):
                sl = slice(nb * IBLK, (nb + 1) * IBLK)
                ps = ps_mlp.tile([128, IBLK], f32)
                nc.tensor.matmul(
                    ps[:], wk_sb[:, NLIN - 1, :], hk[:, sl], start=True, stop=True
                )
                psv = ps[:].rearrange("p (a e) -> p a e", e=64)
                pair = slice(8 * nb, 8 * nb + 8)
                bias = bk_sb[:, NLIN - 1 : NLIN]
                for half, csl in ((slice(0, 64), slice(0, 64)),
                                  (slice(64, 128), slice(64, 128))):
                    dst = kTblk[half, pair, csl]
                    src = psv[half, :, :]
                    if eng_flip % 2 == 0:
                        nc.scalar.activation(dst, src, Identity, bias=bias[half])
                    else:
                        nc.vector.tensor_scalar(dst, src, bias[half], None, op0=add)
                    eng_flip += 1

            # final Q layer: replicate Q^T onto both partition halves
            for nb in range(NQ // 2 // IBLK):
                sl = slice(nb * IBLK, (nb + 1) * IBLK)
                bias = bq_sb[:, NLIN - 1 : NLIN]
                for rep in range(2):
                    ps = ps_mlp.tile([128, IBLK], f32)
                    nc.tensor.matmul(
                        ps[:], wq_sb[:, 2 + rep, :], hq[:, sl], start=True, stop=True
                    )
                    osl = slice(rep * (NQ // 2) + nb * IBLK,
                                rep * (NQ // 2) + (nb + 1) * IBLK)
                    epilogue(qT2[:, osl], ps[:], bias, NLIN - 1,
                             "act" if rep else "dve")

            # ---- main loop: S^T pair = kTblk[jt].T @ qT2 ; mask ; AV
            def emit_av(ov, s_sb, b, jt):
                nc.tensor.matmul(
                    ov[:], xkv_sb[:, jt, :], s_sb[:],
                    start=(jt == 0), stop=(jt == J[b] - 1),
                    skip_group_check=True,
                )
                if jt == J[b] - 1:
                    isl = slice(b * IBLK, (b + 1) * IBLK)
                    nc.scalar.copy(out_sb[:, isl], ov[:])

            alt = 0
            prev = None
            for b in range(NBLK):
                isl = slice(b * IBLK, (b + 1) * IBLK)
                ov = ps_o.tile([2, IBLK], f32)
                for jt in range(J[b]):
                    sp = ps_s.tile([CHUNK, IBLK], f32)
                    nc.tensor.matmul(
                        sp[:], kTblk[:, jt, :], qT2[:, isl],
                        start=True, stop=True, skip_group_check=True,
                    )
                    s_sb = spool.tile([CHUNK, IBLK], f32r)
                    if jt < F[b]:
                        # fully visible: plain copy, mostly on ACT
                        if alt % 3 == 2:
                            nc.vector.tensor_copy(s_sb[:], sp[:])
                        else:
                            nc.scalar.copy(s_sb[:], sp[:])
                        alt += 1
                    else:
                        mk = mpool.tile([CHUNK, IBLK], f32)
                        nc.vector.tensor_scalar(
                            mk[:], t1b_sb[:, isl], xt2_sb[:, jt : jt + 1], None,
                            op0=is_ge,
                        )
                        nc.vector.tensor_mul(s_sb[:], sp[:], mk[:])
                    if prev is not None:
                        emit_av(*prev)
                    prev = (ov, s_sb, b, jt)
            emit_av(*prev)

            nc.sync.dma_start(out[:], out_sb[:])

    nc.compile()
    return nc


def _stack_keys(a):
    """[T, ...] -> even/odd 64-chunk split stacked on a new leading axis."""
    v = a.reshape(NPAIR, 2, 64, *a.shape[1:])
    return v[:, 0], v[:, 1]  # each [NPAIR, 64, ...]


def kernel(x1, x2, x3, x4, Wq_w, Wq_b, Wk_w, Wk_b):
    from concourse.bass_utils import run_bass_kernel_spmd

    global LAST_RESULTS

    xs = [np.asarray(a, dtype=np.float32)[0, 0] for a in (x1, x2, x3, x4)]
    Wq_w = np.asarray(Wq_w, dtype=np.float32)
    Wq_b = np.asarray(Wq_b, dtype=np.float32)
    Wk_w = np.asarray(Wk_w, dtype=np.float32)
    Wk_b = np.asarray(Wk_b, dtype=np.float32)

    t1 = xs[0][:, -1]
    t2s = [x[:, -1] for x in xs]

    # ---- universal tile classification (exact, quantified over all cores)
    J, F = [], []
    for b in range(NBLK):
        blk_lo = t1[1024 * b]
        blk_hi = t1[1024 * b + 1023]
        need, full = 0, NPAIR
        for m in range(M):
            nvis = int(np.searchsorted(t2s[m], blk_hi, side="right"))
            nfull = int(np.searchsorted(t2s[m], blk_lo, side="right"))
            need = max(need, -(-nvis // CHUNK))
            full = min(full, nfull // CHUNK)
        J.append(max(need, 1))
        F.append(min(full, max(need, 1)))

    nc = _build_program(J, F)

    # ---- host packing
    perm = np.empty((2, NQ), dtype=np.int64)
    for p in range(2):
        perm[p] = np.concatenate(
            [np.arange(128 * (2 * k + p), 128 * (2 * k + p) + 128) for k in range(16)]
        )

    def blockdiag(Wl):
        b = np.zeros((128, 128), np.float32)
        b[:64, :64] = Wl
        b[64:, 64:] = Wl
        return b

    # Q weights: layers 0,1 blockdiag; final as [[W,W],[0,0]] and [[0,0],[W,W]]
    wq_h = np.zeros((4, 128, 128), np.float32)
    for l in range(NLIN - 1):
        wq_h[l] = blockdiag(Wq_w[l])
    wq_h[2, :64, :64] = Wq_w[2]
    wq_h[2, :64, 64:] = Wq_w[2]
    wq_h[3, 64:, :64] = Wq_w[2]
    wq_h[3, 64:, 64:] = Wq_w[2]
    wq_h = np.ascontiguousarray(wq_h.transpose(1, 0, 2).reshape(128, 4 * 128))
    bq_h = np.tile(Wq_b.T, (2, 1))  # [128, 3]
    bq_h = np.ascontiguousarray(
        np.concatenate([bq_h, bq_h[:, 2:3]], axis=1)
    )  # [128, 4]

    x1T = np.ascontiguousarray(xs[0].T)

    in_maps = []
    for c in range(8):
        m, p = c // 2, c % 2
        xm = xs[m]
        # key-side stacking: even/odd 64-chunks
        ev, od = _stack_keys(xm)  # [NPAIR, 64, D] each
        xkT_h = np.concatenate(
            [
                ev.reshape(T // 2, D).T,   # [64, 2048]
                od.reshape(T // 2, D).T,
            ],
            axis=0,
        )  # [128, 2048]
        xkv_h = np.concatenate(
            [ev[:, :, 0:2], od[:, :, 0:2]], axis=1
        )  # [NPAIR, 128, 2]
        xkv_h = np.ascontiguousarray(xkv_h.transpose(1, 0, 2).reshape(128, NPAIR * 2))
        xt2_h = np.concatenate(
            [ev[:, :, D - 1], od[:, :, D - 1]], axis=1
        ).T  # [128, NPAIR]

        wk_h = np.stack([blockdiag(Wk_w[m][l]) for l in range(NLIN)])
        wk_h = np.ascontiguousarray(wk_h.transpose(1, 0, 2).reshape(128, NLIN * 128))
        bk_h = np.ascontiguousarray(np.tile(Wk_b[m].T, (2, 1)))  # [128, 3]

        # query-side: parity packing then [first half | second half] stacking
        xq = x1T[:, perm[p]]  # [64, 2048]
        xqT_h = np.concatenate([xq[:, : NQ // 2], xq[:, NQ // 2 :]], axis=0)

        in_maps.append(
            {
                "xqT": np.ascontiguousarray(xqT_h),
                "xkT": np.ascontiguousarray(xkT_h),
                "xkv": xkv_h,
                "xt2": np.ascontiguousarray(xt2_h),
                "t1p": np.ascontiguousarray(t1[perm[p]][None, :]),
                "wq": wq_h,
                "bq": bq_h,
                "wk": wk_h,
                "bk": bk_h,
            }
        )

    res = run_bass_kernel_spmd(nc, in_maps, core_ids=list(range(8)))
    LAST_RESULTS = res

    # ---- gather: sum over modalities, unpermute parity chunks, transpose
    acc = np.zeros((2, T), dtype=np.float32)
    for c in range(8):
        m, p = c // 2, c % 2
        acc[:, perm[p]] += res.results[c]["out"]
    return np.ascontiguousarray(acc.T)[None]



# revision 4
# speedup vs baseline: 1.3465x; 1.3465x over previous
"""Trainium2 Bass kernel for masked multi-modal causal dot-product attention.

Computation (reference):
  Q = mlp(x1, Wq)               # (4096, 64), 3 linear layers, relu between
  for m in 0..3:
    K_m = mlp(x_m, Wk[m])       # (4096, 64)
    mask_m[i,j] = t2_m[j] <= t1[i]   (timestamps sorted -> staircase mask)
    acc += ((Q @ K_m.T) * mask_m) @ x_m[:, :2]
  out = acc  # (1, 4096, 2)

Sharding: 8 cores = 4 modalities x 2 contiguous query halves (2048 queries
each). One SPMD program; per-core variation lives in the input tensors.

Key algebraic optimization: for key tiles FULLY visible to a whole query
block, ((Q K^T) * 1) V = Q (K^T V). Per 128-key pair tile j we form
G_j^T = V_j^T K_j (2x64) with two tiny matmuls:
  - probe:  sp = kTblk_j^T @ [I64; I64]  -> K values, keys on partitions
  - reduce: G_j^T = xkv_j^T @ sp         -> PSUM (2, 64)
G tiles are staged to SBUF and DMA'd into a [32, 2, 64] partition-stacked
array; a step-vector matmul (host-built, per-core data) then selects the
prefix sum G_pref_b = sum_{j < F[b]} G_j for each query block -> the whole
fully-visible region costs ONE 512-col matmul per block. Only the ~5 boundary
tiles per block (keys whose timestamp falls inside the block's time span) run
the explicit S -> fused mask-multiply (DVE scalar_tensor_tensor) -> AV path.
Boundary keys are host-gathered into fixed tile slots so a single program
serves all cores; padded slots use t2=+inf and mask to zero.

Packing (from baseline): feature dim 64 is packed to contraction 128
everywhere (block-diagonal MLP weights on stacked halves, block-diagonal
K^T pair tiles, Q^T replicated onto both partition halves). All matmuls f32r.
"""

import os
import sys

import numpy as np

sys.path.insert(0, "/opt/trn_rl_repo")

T = 4096
D = 64
M = 4
NLIN = 3
NQ = 2048          # queries per core (contiguous half)
CHUNK = 128        # keys per pair tile (64 even + 64 odd)
NPAIR = T // CHUNK  # 32 sorted pair tiles
IBLK = 512         # query block (moving dim)
NBLK = NQ // IBLK  # 4 query blocks per core

LAST_RESULTS = None


def _build_program(NBB):
    """NBB[b]: boundary slots for query block b (same for all cores; per-core
    variation is in the gathered input data)."""
    import concourse.bacc as bacc
    import concourse.mybir as mybir
    import concourse.tile as tile

    f32 = mybir.dt.float32
    f32r = mybir.dt.float32r
    Relu = mybir.ActivationFunctionType.Relu
    Identity = mybir.ActivationFunctionType.Identity
    is_ge = mybir.AluOpType.is_ge
    add = mybir.AluOpType.add
    amax = mybir.AluOpType.max
    mult = mybir.AluOpType.mult

    NBSLOT = sum(NBB)             # total boundary slots
    NSLOT = NPAIR + NBSLOT        # total pair tiles in kTblk
    KCOLS = NSLOT * 64            # K-MLP moving columns
    boff = [NPAIR + sum(NBB[:b]) for b in range(NBLK)]  # first slot of block b

    nc = bacc.Bacc("TRN2", target_bir_lowering=False, debug=False, num_devices=8)

    xqT = nc.dram_tensor("xqT", [128, NQ // 2], f32, kind="ExternalInput")
    xkT = nc.dram_tensor("xkT", [128, KCOLS], f32, kind="ExternalInput")
    xkv = nc.dram_tensor("xkv", [128, NSLOT * 2], f32, kind="ExternalInput")
    xt2b = nc.dram_tensor("xt2b", [128, max(NBSLOT, 1)], f32, kind="ExternalInput")
    t1p = nc.dram_tensor("t1p", [1, NQ], f32, kind="ExternalInput")
    probe = nc.dram_tensor("probe", [128, 64], f32, kind="ExternalInput")
    stepm = nc.dram_tensor("stepm", [NPAIR, NBLK], f32, kind="ExternalInput")
    wq = nc.dram_tensor("wq", [128, 4 * 128], f32, kind="ExternalInput")
    bq = nc.dram_tensor("bq", [128, 4], f32, kind="ExternalInput")
    wk = nc.dram_tensor("wk", [128, NLIN * 128], f32, kind="ExternalInput")
    bk = nc.dram_tensor("bk", [128, NLIN], f32, kind="ExternalInput")
    out = nc.dram_tensor("out", [2, NQ], f32, kind="ExternalOutput")

    def rr(ap):
        return ap.bitcast(f32r)

    with tile.TileContext(nc) as tc:
        with (
            tc.tile_pool(name="const", bufs=1) as const,
            tc.tile_pool(name="hq", bufs=2) as hqp,
            tc.tile_pool(name="hk", bufs=2) as hkp,
            tc.tile_pool(name="spool", bufs=3) as spool,
            tc.tile_pool(name="gpool", bufs=3) as gpool,
            tc.tile_pool(name="gstg", bufs=2) as gstg,
            tc.tile_pool(name="ps_a", bufs=3, space="PSUM") as ps_a,
            tc.tile_pool(name="ps_s", bufs=3, space="PSUM") as ps_s,
            tc.tile_pool(name="ps_og", bufs=2, space="PSUM") as ps_og,
        ):
            # ---- inputs -> SBUF (weights first, x chunked for overlap)
            wq_sb = const.tile([128, 4, 128], f32r)
            nc.sync.dma_start(wq_sb[:], rr(wq[:]).rearrange("p (l e) -> p l e", l=4))
            bq_sb = const.tile([128, 4], f32)
            nc.sync.dma_start(bq_sb[:], bq[:])
            wk_sb = const.tile([128, NLIN, 128], f32r)
            nc.sync.dma_start(wk_sb[:], rr(wk[:]).rearrange("p (l e) -> p l e", l=NLIN))
            bk_sb = const.tile([128, NLIN], f32)
            nc.sync.dma_start(bk_sb[:], bk[:])
            xkv_sb = const.tile([128, NSLOT, 2], f32r)
            nc.sync.dma_start(xkv_sb[:], rr(xkv[:]).rearrange("p (c f) -> p c f", f=2))
            xt2b_sb = const.tile([128, max(NBSLOT, 1)], f32)
            nc.sync.dma_start(xt2b_sb[:], xt2b[:])
            probe_sb = const.tile([128, 64], f32r)
            nc.sync.dma_start(probe_sb[:], rr(probe[:]))
            step_sb = const.tile([NPAIR, NBLK], f32r)
            nc.sync.dma_start(step_sb[:], rr(stepm[:]))
            t1b_sb = const.tile([CHUNK, NQ], f32)
            nc.sync.dma_start(t1b_sb[:], t1p[:].partition_broadcast(CHUNK))

            xqT_sb = const.tile([128, NQ // 2], f32r)
            for nb in range(NQ // 2 // IBLK):
                sl = slice(nb * IBLK, (nb + 1) * IBLK)
                nc.sync.dma_start(xqT_sb[:, sl], rr(xqT[:, sl]))
            xkT_sb = const.tile([128, KCOLS], f32r)
            nchk = -(-KCOLS // IBLK)
            for nb in range(nchk):
                sl = slice(nb * IBLK, min((nb + 1) * IBLK, KCOLS))
                nc.sync.dma_start(xkT_sb[:, sl], rr(xkT[:, sl]))

            out_sb = const.tile([2, NQ], f32)

            # ---- blocked K^T target: pair tiles with block-diagonal layout
            kTblk = const.tile([128, NSLOT, CHUNK], f32r)
            nc.vector.memset(kTblk[0:64, :, 64:128].bitcast(f32), 0.0)
            nc.gpsimd.memset(kTblk[64:128, :, 0:64].bitcast(f32), 0.0)
            qT2 = const.tile([128, NQ], f32r)
            G_stack = const.tile([NPAIR, 2, 64], f32r)
            gstat = const.tile([128, NBLK, 2], f32r)
            nc.gpsimd.memset(gstat[:].bitcast(f32), 0.0)

            # ---- stacked MLPs (block-diagonal weights, both halves at once)
            def epilogue(dst, ps, bias, layer, eng):
                if eng == "act":
                    func = Relu if layer < NLIN - 1 else Identity
                    nc.scalar.activation(dst, ps, func, bias=bias)
                elif layer < NLIN - 1:
                    nc.vector.tensor_scalar(dst, ps, bias, 0.0, op0=add, op1=amax)
                else:
                    nc.vector.tensor_scalar(dst, ps, bias, None, op0=add)

            def mlp_hidden(cur, w_sb, b_sb, pool, nt, layer, eng):
                nxt = pool.tile([128, nt], f32r, tag="h")
                for nb in range(-(-nt // IBLK)):
                    sl = slice(nb * IBLK, min((nb + 1) * IBLK, nt))
                    csz = sl.stop - sl.start
                    ps = ps_a.tile([128, csz], f32, tag="a")
                    nc.tensor.matmul(
                        ps[:], w_sb[:, layer, :], cur[:, sl], start=True, stop=True
                    )
                    epilogue(nxt[:, sl], ps[:], b_sb[:, layer : layer + 1], layer, eng)
                return nxt

            hk, hq = xkT_sb, xqT_sb
            for layer in range(NLIN - 1):
                hk = mlp_hidden(hk, wk_sb, bk_sb, hkp, KCOLS, layer, "act")
                hq = mlp_hidden(hq, wq_sb, bq_sb, hqp, NQ // 2, layer, "dve")

            # final K layer: write straight into block-diagonal pair tiles
            eng_flip = 0
            for nb in range(nchk):
                sl = slice(nb * IBLK, min((nb + 1) * IBLK, KCOLS))
                csz = sl.stop - sl.start
                ps = ps_a.tile([128, csz], f32, tag="a")
                nc.tensor.matmul(
                    ps[:], wk_sb[:, NLIN - 1, :], hk[:, sl], start=True, stop=True
                )
                psv = ps[:].rearrange("p (a e) -> p a e", e=64)
                pair = slice(8 * nb, 8 * nb + csz // 64)
                bias = bk_sb[:, NLIN - 1 : NLIN]
                for half, csl in ((slice(0, 64), slice(0, 64)),
                                  (slice(64, 128), slice(64, 128))):
                    dst = kTblk[half, pair, csl]
                    src = psv[half, :, :]
                    if eng_flip % 2 == 0:
                        nc.scalar.activation(dst, src, Identity, bias=bias[half])
                    else:
                        nc.vector.tensor_scalar(dst, src, bias[half], None, op0=add)
                    eng_flip += 1

            # ---- G phase: G_j^T = V_j^T K_j via probe matmuls, 8 tiles/round
            gps = None
            for j in range(NPAIR):
                r, slq = divmod(j, 8)
                if slq == 0:
                    gps = ps_og.tile([2, 512], f32, tag="og")
                sp = ps_a.tile([128, 64], f32, tag="a")
                nc.tensor.matmul(
                    sp[:], kTblk[:, j, :], probe_sb[:], start=True, stop=True,
                    skip_group_check=True,
                )
                sps = gpool.tile([128, 64], f32r)
                nc.scalar.copy(sps[:], sp[:])
                nc.tensor.matmul(
                    gps[:, slq * 64:(slq + 1) * 64], xkv_sb[:, j, :], sps[:],
                    start=True, stop=True, skip_group_check=True,
                )
                if slq == 7:
                    gst = gstg.tile([2, 512], f32r)
                    nc.vector.tensor_copy(gst[:], gps[:])
                    for c in range(2):
                        nc.sync.dma_start(
                            G_stack[r * 8:(r + 1) * 8, c, :], gst[c:c + 1, :]
                        )

            # final Q layer: replicate Q^T onto both partition halves
            # (emitted after probes to keep PE busy while G DMAs land)
            for nb in range(NQ // 2 // IBLK):
                sl = slice(nb * IBLK, (nb + 1) * IBLK)
                bias = bq_sb[:, NLIN - 1 : NLIN]
                for rep in range(2):
                    ps = ps_a.tile([128, IBLK], f32, tag="a")
                    nc.tensor.matmul(
                        ps[:], wq_sb[:, 2 + rep, :], hq[:, sl], start=True, stop=True
                    )
                    osl = slice(rep * (NQ // 2) + nb * IBLK,
                                rep * (NQ // 2) + (nb + 1) * IBLK)
                    epilogue(qT2[:, osl], ps[:], bias, NLIN - 1,
                             "act" if rep else "dve")

            # ---- prefix select: G_pref_b = sum_{j < F[b]} G_j (step is data!)
            psel = ps_s.tile([64, 2 * NBLK], f32, tag="s")
            for c in range(2):
                nc.tensor.matmul(
                    psel[:, c * NBLK:(c + 1) * NBLK], G_stack[:, c, :], step_sb[:],
                    start=True, stop=True, skip_group_check=True,
                )
            for b in range(NBLK):
                for c in range(2):
                    i = c * NBLK + b
                    nc.scalar.copy(gstat[0:64, b, c:c + 1], psel[:, i:i + 1])

            # ---- main loop: one full-region matmul + NBB[b] boundary tiles
            def emit_av(ov, s_sb, slot, last, isl):
                nc.tensor.matmul(
                    ov[:], xkv_sb[:, slot, :], s_sb[:],
                    start=False, stop=last, skip_group_check=True,
                )
                if last:
                    nc.scalar.copy(out_sb[:, isl], ov[:])

            prev = None
            for b in range(NBLK):
                isl = slice(b * IBLK, (b + 1) * IBLK)
                ov = ps_og.tile([2, IBLK], f32, tag="og")
                nc.tensor.matmul(
                    ov[:], gstat[:, b, :], qT2[:, isl],
                    start=True, stop=False, skip_group_check=True,
                )
                for s in range(NBB[b]):
                    slot = boff[b] + s
                    bidx = slot - NPAIR
                    sp = ps_s.tile([CHUNK, IBLK], f32, tag="s")
                    nc.tensor.matmul(
                        sp[:], kTblk[:, slot, :], qT2[:, isl],
                        start=True, stop=True, skip_group_check=True,
                    )
                    s_sb = spool.tile([CHUNK, IBLK], f32r)
                    nc.vector.scalar_tensor_tensor(
                        s_sb[:], t1b_sb[:, isl], xt2b_sb[:, bidx:bidx + 1], sp[:],
                        op0=is_ge, op1=mult,
                    )
                    if prev is not None:
                        emit_av(*prev)
                    prev = (ov, s_sb, slot, s == NBB[b] - 1, isl)
            emit_av(*prev)

            nc.sync.dma_start(out[:], out_sb[:])

    nc.compile()
    return nc


def kernel(x1, x2, x3, x4, Wq_w, Wq_b, Wk_w, Wk_b):
    from concourse.bass_utils import run_bass_kernel_spmd

    global LAST_RESULTS

    xs = [np.asarray(a, dtype=np.float32)[0, 0] for a in (x1, x2, x3, x4)]
    Wq_w = np.asarray(Wq_w, dtype=np.float32)
    Wq_b = np.asarray(Wq_b, dtype=np.float32)
    Wk_w = np.asarray(Wk_w, dtype=np.float32)
    Wk_b = np.asarray(Wk_b, dtype=np.float32)

    t1 = xs[0][:, -1]
    t2s = [x[:, -1] for x in xs]

    # ---- per-core full/boundary classification (exact, from timestamps)
    FJ = {}  # (m, p) -> (F[b], J[b])
    NBB = [1] * NBLK
    for p in range(2):
        qoff = NQ * p
        for m in range(M):
            F, J = [], []
            for b in range(NBLK):
                lo = t1[qoff + b * IBLK]
                hi = t1[qoff + b * IBLK + IBLK - 1]
                nfull = int(np.searchsorted(t2s[m], lo, side="right"))
                nvis = int(np.searchsorted(t2s[m], hi, side="right"))
                F.append(nfull // CHUNK)
                J.append(-(-nvis // CHUNK))
                NBB[b] = max(NBB[b], J[b] - F[b])
            FJ[(m, p)] = (F, J)

    nc = _build_program(NBB)

    NBSLOT = sum(NBB)
    boff = [sum(NBB[:b]) for b in range(NBLK)]

    # ---- host packing
    def blockdiag(Wl):
        b = np.zeros((128, 128), np.float32)
        b[:64, :64] = Wl
        b[64:, 64:] = Wl
        return b

    # Q weights: layers 0,1 blockdiag; final as [[W,W],[0,0]] and [[0,0],[W,W]]
    wq_h = np.zeros((4, 128, 128), np.float32)
    for l in range(NLIN - 1):
        wq_h[l] = blockdiag(Wq_w[l])
    wq_h[2, :64, :64] = Wq_w[2]
    wq_h[2, :64, 64:] = Wq_w[2]
    wq_h[3, 64:, :64] = Wq_w[2]
    wq_h[3, 64:, 64:] = Wq_w[2]
    wq_h = np.ascontiguousarray(wq_h.transpose(1, 0, 2).reshape(128, 4 * 128))
    bq_h = np.tile(Wq_b.T, (2, 1))  # [128, 3]
    bq_h = np.ascontiguousarray(
        np.concatenate([bq_h, bq_h[:, 2:3]], axis=1)
    )  # [128, 4]

    probe_h = np.ascontiguousarray(
        np.concatenate([np.eye(64, dtype=np.float32)] * 2, axis=0)
    )  # [128, 64]

    x1T = np.ascontiguousarray(xs[0].T)

    def pack_tile(xrows):
        """[128, D] key rows -> ([128, 64] xkT block, [128, 2] V, [128] t2)."""
        ev, od = xrows[0:64], xrows[64:128]
        blk = np.concatenate([ev.T, od.T], axis=0)  # [128, 64]
        v = np.concatenate([ev[:, 0:2], od[:, 0:2]], axis=0)  # [128, 2]
        tt = np.concatenate([ev[:, -1], od[:, -1]], axis=0)  # [128]
        return blk, v, tt

    in_maps = []
    for c in range(8):
        m, p = c // 2, c % 2
        xm = xs[m]
        qoff = NQ * p
        F, J = FJ[(m, p)]

        NSLOT = NPAIR + NBSLOT
        xkT_h = np.zeros((128, NSLOT * 64), np.float32)
        xkv_h = np.zeros((128, NSLOT, 2), np.float32)
        xt2b_h = np.full((128, max(NBSLOT, 1)), 1e30, np.float32)
        for j in range(NPAIR):
            blk, v, tt = pack_tile(xm[CHUNK * j:CHUNK * (j + 1)])
            xkT_h[:, 64 * j:64 * (j + 1)] = blk
            xkv_h[:, j] = v
        for b in range(NBLK):
            for s in range(NBB[b]):
                t = F[b] + s
                slot = NPAIR + boff[b] + s
                if t < J[b]:
                    blk, v, tt = pack_tile(xm[CHUNK * t:CHUNK * (t + 1)])
                    xkT_h[:, 64 * slot:64 * (slot + 1)] = blk
                    xkv_h[:, slot] = v
                    xt2b_h[:, boff[b] + s] = tt
        step_h = np.zeros((NPAIR, NBLK), np.float32)
        for b in range(NBLK):
            step_h[: F[b], b] = 1.0

        wk_h = np.stack([blockdiag(Wk_w[m][l]) for l in range(NLIN)])
        wk_h = np.ascontiguousarray(wk_h.transpose(1, 0, 2).reshape(128, NLIN * 128))
        bk_h = np.ascontiguousarray(np.tile(Wk_b[m].T, (2, 1)))  # [128, 3]

        # query-side: contiguous half, [first 1024 | second 1024] stacking
        xq = x1T[:, qoff:qoff + NQ]  # [64, 2048]
        xqT_h = np.concatenate([xq[:, : NQ // 2], xq[:, NQ // 2:]], axis=0)

        in_maps.append(
            {
                "xqT": np.ascontiguousarray(xqT_h),
                "xkT": xkT_h,
                "xkv": np.ascontiguousarray(xkv_h.reshape(128, NSLOT * 2)),
                "xt2b": xt2b_h,
                "t1p": np.ascontiguousarray(t1[qoff:qoff + NQ][None, :]),
                "probe": probe_h,
                "stepm": step_h,
                "wq": wq_h,
                "bq": bq_h,
                "wk": wk_h,
                "bk": bk_h,
            }
        )

    res = run_bass_kernel_spmd(nc, in_maps, core_ids=list(range(8)))
    LAST_RESULTS = res

    # ---- gather: sum over modalities per contiguous half, transpose
    acc = np.zeros((2, T), dtype=np.float32)
    for c in range(8):
        m, p = c // 2, c % 2
        acc[:, NQ * p:NQ * (p + 1)] += res.results[c]["out"]
    return np.ascontiguousarray(acc.T)[None]
